# revision 64
# baseline (speedup 1.0000x reference)
"""Plane-sweep cost-volume kernel for Trainium2 (8 NeuronCores).

Problem shape (hardcoded): B=1, V=4 source views, C=16 feature channels,
H=64, W=96, D=64 depth planes.  Output: (1, D, H, W) float32.

Strategy
--------
The benchmark geometry has identity rotations (extrinsics are pure
translations) and zero-skew pinhole intrinsics, so for each (view, depth
plane) the warp from output pixels to source-image sample coordinates is an
axis-separable affine map.  Bilinear grid_sample with zero padding then
factorizes exactly into two 1-D linear interpolations, each a small dense
matrix of "hat" functions hat(t - k) = max(0, 1 - |t - k|):

    warped_c = Ay(v,d) @ src_c @ Bx(v,d)^T

so the whole cost volume becomes TensorEngine matmuls — no gathers.  The
view sum is accumulated in PSUM, and the channel dot with cur_feats is a
vector multiply + reduce.

Optimizations vs the first working version (101.7us -> ~58us measured):
  * Ay/Bx hat matrices + bf16 features precomputed on host and DMAed on the
    two HWDGE queues, biggest-consumer-first (removes ~25us of on-device
    setup that serialized in front of the matmuls).
  * Stage-1 contraction zero-padded from K=64 to K=96 rows: with only half
    the PE array active, the PE_HAM activity monitor never un-throttles the
    2.4 GHz clock gate (stage 1 measured a flat 1.2 GHz); at K=96 it warms
    after ~3.4us.  Seven dummy matmuls on a scratch tile pre-warm the PE
    during the input-DMA wait.
  * The 3.1M-element stage-1 PSUM->SBUF intermediate is the hard wall (only
    vector + scalar can read PSUM, ~1.1-1.2us per 1024-elem chunk each);
    casts alternate engines, 4-deep PSUM buffering.
  * Channel dot: scalar (idle in stage 2) copies PSUM->SBUF bf16 so the
    multiply runs in the DVE 2x bf16 mode; reduction is a vector X-reduce or
    a vector+gpsimd hybrid add-tree per REDUCE_PATTERN (empirically tuned -
    moving the last plane's reduce to vector measures ~10us slower).

Sharding: depth planes across the 8 cores (8 planes each); features are
replicated.  If the inputs do not have the separable structure, we fall
back to an exact numpy implementation.
"""

import numpy as np
import ml_dtypes

H, W, D, V, C = 64, 96, 64, 4, 16
N_CORES = 8
DLOC = D // N_CORES            # 8 depth planes per core
EPS = 1e-8
OOB = 1.0e9                    # sample coord pushed out of range => zero weights

# cast-engine pattern for the 32 stage-1 cast chunks: 's'calar / 'v'ector
# (scalar is slightly faster per chunk, so it takes 17 of 32)
CAST_PATTERN = "svsvsvsvsvsvsvsssvsvsvsvsvsvsvsv"
# reduce-engine pattern for the 8 plane dots: 'v'ector / 'g'psimd
REDUCE_PATTERN = "gggggvvg"
# stage-1 contraction rows: H=64 padded with zero rows to 96.  With only 64
# of 128 PE rows active the PE_HAM activity monitor never un-throttles the
# clock gate (stage 1 measured a flat 1.2 GHz for 34 us); at K=96 the array
# reads as busy and reaches 2.4 GHz after ~3.4 us.
KPAD = 96
# Stage-1 matmuls convoy at cast speed (frequent small stalls), which keeps
# the PE HAM warm through the stage boundary on its own — no dummy matmuls
# needed (they would just delay stage 2 in the in-order queue).

_CACHE = {}


# --------------------------------------------------------------------------
# Device kernel
# --------------------------------------------------------------------------
def _build_nc():
    import concourse.bacc as bacc
    import concourse.tile as tile
    from concourse import mybir

    fp32 = mybir.dt.float32
    bf16 = mybir.dt.bfloat16
    Alu = mybir.AluOpType
    Axis = mybir.AxisListType

    nc = bacc.Bacc("TRN2", target_bir_lowering=False, debug=False,
                   num_devices=N_CORES)

    srcw = nc.dram_tensor("srcw", [H, V, C, W], bf16, kind="ExternalInput")
    ay = nc.dram_tensor("ay", [KPAD, V * DLOC * H], bf16, kind="ExternalInput")
    bx = nc.dram_tensor("bx", [W, V * DLOC * W], bf16, kind="ExternalInput")
    curb = nc.dram_tensor("curb", [W, C, H], bf16, kind="ExternalInput")
    out = nc.dram_tensor("out", [W, DLOC, H], fp32, kind="ExternalOutput")

    with tile.TileContext(nc) as tc:
        with (
            tc.tile_pool(name="consts", bufs=1) as consts,
            tc.tile_pool(name="tp", bufs=1) as tp_pool,
            tc.tile_pool(name="dot", bufs=3) as dot_pool,
            tc.tile_pool(name="osb", bufs=3) as out_pool,
        ):
            # ---- load constants (all pre-packed on host) ------------------
            # HWDGE queues only (sync + scalar), ordered so the view-0
            # operands land first and stage 1 can start ASAP.  Partition rows
            # H..KPAD-1 of the stage-1 operands are zero (see KPAD note).
            # scratch operand for PE warm-up matmuls (memset so the race
            # detector sees it initialized; values are never read)
            warmg = consts.tile([KPAD, DLOC * H], bf16, tag="warmg")
            nc.vector.memset(warmg, 0.0)
            src_all = consts.tile([KPAD, V, C, W], bf16, tag="src_all")
            # zero the K-pad rows with the two otherwise-idle engines while
            # the input DMAs run (the pad must be exactly zero: it multiplies
            # the Ay pad rows, which are zero via the host, but 0*garbage
            # could be NaN if this region held Inf/NaN bits)
            nc.vector.memset(src_all[H:KPAD, 0:2, :, :], 0.0)
            nc.gpsimd.memset(src_all[H:KPAD, 2:4, :, :], 0.0)
            nc.sync.dma_start(out=src_all[0:H, 0:1, :, :],
                              in_=srcw.ap()[:, 0:1, :, :])
            Ay = consts.tile([KPAD, V * DLOC * H], bf16, tag="ay")
            nc.scalar.dma_start(out=Ay[:, 0:DLOC * H],
                                in_=ay.ap()[:, 0:DLOC * H])
            nc.sync.dma_start(out=src_all[0:H, 1:2, :, :],
                              in_=srcw.ap()[:, 1:2, :, :])
            nc.scalar.dma_start(out=Ay[:, DLOC * H:],
                                in_=ay.ap()[:, DLOC * H:])
            nc.sync.dma_start(out=src_all[0:H, 2:4, :, :],
                              in_=srcw.ap()[:, 2:4, :, :])
            Bx = consts.tile([W, V * DLOC * W], bf16, tag="bx")
            nc.scalar.dma_start(out=Bx, in_=bx.ap())
            cur_t = consts.tile([W, C, H], bf16, tag="cur")
            nc.scalar.dma_start(out=cur_t, in_=curb.ap())
            src_t = [src_all[:, v, :, :] for v in range(V)]

            tps = [tp_pool.tile([W, C, DLOC * H], bf16, tag=f"tp{v}",
                                name=f"tp{v}")
                   for v in range(V)]

            # ---- stage 1: y-interpolation -------------------------------
            # tp_v(w; c, (d,py)) = sum_h src(h; c, w) * Ay(h; v, (d,py))
            ci = 0
            with tc.tile_pool(name="ps1", bufs=4, space="PSUM") as ps1_pool:
                # warm the PE HAM clock gate during the input-DMA wait: these
                # depend on nothing, so by the time the real operands land the
                # PE is already at 2.4 GHz instead of spending its first
                # ~3.4 us of real work at 1.2 GHz.
                ps1 = ps1_pool.tile([W, 2, DLOC * H], fp32)
                for _ in range(9):
                    nc.tensor.matmul(ps1[:, 0, :], warmg[:, 0:W], warmg,
                                     start=True, stop=True)
                for v in range(V):
                    rhs = Ay[:, v * DLOC * H:(v + 1) * DLOC * H]   # (64, 512)
                    for cq in range(C // 2):
                        ps1 = ps1_pool.tile([W, 2, DLOC * H], fp32)
                        for cc in range(2):
                            nc.tensor.matmul(
                                ps1[:, cc, :], src_t[v][:, cq * 2 + cc, :],
                                rhs, start=True, stop=True)
                        dst = tps[v][:, cq * 2:cq * 2 + 2, :]
                        if CAST_PATTERN[ci % len(CAST_PATTERN)] == 's':
                            nc.scalar.copy(dst, ps1)
                        else:
                            nc.vector.tensor_copy(dst, ps1)
                        ci += 1

            # ---- stage 2: x-interpolation + view accumulation -----------
            osb_all = out_pool.tile([W, DLOC, H], fp32, tag="osb_all")
            ps2_cm = tc.tile_pool(name="ps2", bufs=3, space="PSUM")
            ps2_pool = ps2_cm.__enter__()
            for d in range(DLOC):
                ps2 = ps2_pool.tile([W, C, H], fp32)
                for v in range(V):
                    lhsT = Bx[:, (v * DLOC + d) * W:(v * DLOC + d + 1) * W]
                    for hh in range(2):
                        nc.tensor.matmul(
                            ps2[:, hh * 8:hh * 8 + 8, :],
                            lhsT,
                            tps[v][:, hh * 8:hh * 8 + 8, d * H:(d + 1) * H],
                            start=(v == 0), stop=(v == V - 1))
                # channel dot with cur + write out.  For all but the last
                # plane, the (otherwise idle) scalar engine moves ps2 out of
                # PSUM so the multiply runs in the DVE 2x bf16 SBUF mode; the
                # last plane multiplies straight from PSUM to shorten the
                # tail dependency chain.
                m = dot_pool.tile([W, C, H], bf16)
                if d < DLOC - 1:
                    p2b = dot_pool.tile([W, C, H], bf16)
                    nc.scalar.copy(p2b, ps2)
                    nc.vector.tensor_mul(m, p2b, cur_t)
                else:
                    nc.vector.tensor_mul(m, ps2, cur_t)
                osb = osb_all[:, d, :]
                if REDUCE_PATTERN[d % len(REDUCE_PATTERN)] == 'v':
                    nc.vector.tensor_reduce(
                        osb, m.transpose([0, 2, 1]), axis=Axis.X, op=Alu.add)
                else:
                    # gpsimd can't X-reduce; binary tree of adds over C.
                    # The first (largest) level runs on vector in its 2x bf16
                    # mode; the small tail levels go to gpsimd (~2.1 cyc/elem).
                    s1 = dot_pool.tile([W, C // 2, H], bf16)
                    nc.vector.tensor_add(s1, m[:, :8, :], m[:, 8:, :])
                    s2 = dot_pool.tile([W, C // 4, H], bf16)
                    nc.gpsimd.tensor_add(s2, s1[:, :4, :], s1[:, 4:, :])
                    s3 = dot_pool.tile([W, C // 8, H], bf16)
                    nc.gpsimd.tensor_add(s3, s2[:, :2, :], s2[:, 2:, :])
                    nc.gpsimd.tensor_add(osb, s3[:, 0, :], s3[:, 1, :])
                if d == 3:
                    nc.sync.dma_start(out=out.ap()[:, 0:4, :],
                                      in_=osb_all[:, 0:4, :])
                elif d == 6:
                    nc.sync.dma_start(out=out.ap()[:, 4:7, :],
                                      in_=osb_all[:, 4:7, :])
                elif d == 7:
                    nc.sync.dma_start(out=out.ap()[:, 7:8, :],
                                      in_=osb_all[:, 7:8, :])
            ps2_cm.__exit__(None, None, None)

    nc.compile()
    return nc


def _get_nc():
    if "nc" not in _CACHE:
        _CACHE["nc"] = _build_nc()
    return _CACHE["nc"]


# --------------------------------------------------------------------------
# Host-side geometry
# --------------------------------------------------------------------------
def _depth_planes(min_depth, max_depth):
    """Mimic the reference's fp32 arithmetic."""
    ramp = np.linspace(0.0, 1.0, D, dtype=np.float32)
    inv_min = (np.float32(1.0) / np.float32(min_depth)).astype(np.float32)
    inv_max = (np.float32(1.0) / np.float32(max_depth)).astype(np.float32)
    return (np.float32(1.0) /
            (inv_min + (inv_max - inv_min) * ramp).astype(np.float32))


def _is_separable(src_extrinsics, src_Ks, cur_invK):
    E = src_extrinsics[0]          # (V,4,4)
    K = src_Ks[0]                  # (V,4,4)
    iK = cur_invK[0]               # (4,4)
    eye3 = np.eye(3, dtype=E.dtype)
    for v in range(V):
        if not np.array_equal(E[v, :3, :3], eye3):
            return False
        if not np.array_equal(E[v, 3], np.array([0, 0, 0, 1], dtype=E.dtype)):
            return False
        k = K[v]
        if not (k[0, 1] == 0 and k[0, 3] == 0 and k[1, 0] == 0 and k[1, 3] == 0
                and np.array_equal(k[2], np.array([0, 0, 1, 0], dtype=K.dtype))):
            return False
    if not (iK[0, 1] == 0 and iK[1, 0] == 0 and iK[2, 0] == 0
            and iK[2, 1] == 0 and iK[2, 2] == 1):
        return False
    return True


def _coords(src_extrinsics, src_Ks, cur_invK, depths):
    """Per-(view, plane) 1-D sample coordinates: x[v,d,px], y[v,d,py]."""
    E = src_extrinsics[0].astype(np.float64)
    K = src_Ks[0].astype(np.float64)
    iK = cur_invK[0].astype(np.float64)
    i00, i02 = iK[0, 0], iK[0, 2]
    i11, i12 = iK[1, 1], iK[1, 2]
    px = np.arange(W, dtype=np.float64) + 0.5
    py = np.arange(H, dtype=np.float64) + 0.5
    xcs = np.empty((V, D, W), np.float64)
    ycs = np.empty((V, D, H), np.float64)
    for v in range(V):
        k00, k02 = K[v, 0, 0], K[v, 0, 2]
        k11, k12 = K[v, 1, 1], K[v, 1, 2]
        tx, ty, tz = E[v, 0, 3], E[v, 1, 3], E[v, 2, 3]
        for d in range(D):
            Dd = float(depths[d])
            z32 = np.float32(depths[d]) + np.float32(tz)        # ref fp32 z
            if not (z32 > 0):
                xcs[v, d] = OOB
                ycs[v, d] = OOB
                continue
            Zs = float(np.float32(z32 + np.float32(EPS)))
            rx = i00 * px + i02
            ry = i11 * py + i12
            u = (k00 * rx * Dd + k02 * Dd + k00 * tx + k02 * tz) / Zs
            vv = (k11 * ry * Dd + k12 * Dd + k11 * ty + k12 * tz) / Zs
            xcs[v, d] = np.clip(np.nan_to_num(u - 0.5, nan=OOB,
                                              posinf=OOB, neginf=-OOB),
                                -OOB, OOB)
            ycs[v, d] = np.clip(np.nan_to_num(vv - 0.5, nan=OOB,
                                              posinf=OOB, neginf=-OOB),
                                -OOB, OOB)
    return xcs, ycs


def _hat(coords, n, npad=None):
    """coords: (V, DLOC, M) sample positions -> (npad, V*DLOC*M) bf16 hat
    matrix, hat[k, (v,d,m)] = relu(1 - |coords[v,d,m] - k|), zero rows
    beyond n."""
    kk = np.arange(n, dtype=np.float64)
    h = np.maximum(0.0, 1.0 - np.abs(coords[..., None, :] -
                                     kk[None, None, :, None]))
    # h: (V, DLOC, n_k, M) -> (n_k, V, DLOC, M)
    h = np.ascontiguousarray(h.transpose(2, 0, 1, 3)).reshape(n, -1)
    if npad is not None and npad > n:
        h = np.concatenate([h, np.zeros((npad - n, h.shape[1]), h.dtype)], 0)
    return h.astype(ml_dtypes.bfloat16)


# --------------------------------------------------------------------------
# Exact numpy fallback (general geometry)
# --------------------------------------------------------------------------
def _reference_numpy(cur_feats, src_feats, src_extrinsics, src_Ks, cur_invK,
                     min_depth, max_depth):
    f32 = np.float32
    N = H * W
    dp = _depth_planes(min_depth.reshape(-1)[0], max_depth.reshape(-1)[0])
    xx, yy = np.meshgrid(np.arange(W, dtype=f32) + 0.5,
                         np.arange(H, dtype=f32) + 0.5)
    pix = np.stack([xx.ravel(), yy.ravel(), np.ones(N, f32)], 0)       # (3,N)
    rays = cur_invK[0, :3, :3].astype(f32) @ pix                       # (3,N)
    world = rays[None] * dp[:, None, None]                             # (D,3,N)
    world4 = np.concatenate([world, np.ones((D, 1, N), f32)], 1)       # (D,4,N)
    P = np.einsum("vij,vjk->vik", src_Ks[0], src_extrinsics[0])[:, :3]  # (V,3,4)
    cam = np.einsum("vij,djn->vdin", P, world4).astype(f32)            # (V,D,3,N)
    z = cam[:, :, 2]
    u = cam[:, :, 0] / (z + f32(EPS))
    vv = cam[:, :, 1] / (z + f32(EPS))
    x = (u - 0.5).astype(f32).reshape(V, D * N)
    y = (vv - 0.5).astype(f32).reshape(V, D * N)
    out = np.zeros((D, H, W), f32)
    cur = cur_feats[0].reshape(C, N)                                   # (C,N)
    for v in range(V):
        f = src_feats[0, v].reshape(C, N)
        x0 = np.floor(x[v])
        y0 = np.floor(y[v])
        acc = np.zeros((C, D * N), f32)
        for dx in (0.0, 1.0):
            for dy in (0.0, 1.0):
                xi = x0 + dx
                yi = y0 + dy
                wgt = (1.0 - np.abs(x[v] - xi)) * (1.0 - np.abs(y[v] - yi))
                valid = ((xi >= 0) & (xi < W) & (yi >= 0) & (yi < H))
                idx = (np.clip(yi, 0, H - 1) * W +
                       np.clip(xi, 0, W - 1)).astype(np.int64)
                acc += f[:, idx] * (wgt * valid.astype(f32))[None]
        dot = (acc.reshape(C, D, N) *
               cur[:, None, :]).sum(0)                                 # (D,N)
        mask = (z[v] > 0).astype(f32)                                  # (D,N)
        out += (dot * mask).reshape(D, H, W)
    return out[None].astype(np.float32)


# --------------------------------------------------------------------------
# Entry points
# --------------------------------------------------------------------------
def _prepare_inputs(cur_feats, src_feats, src_extrinsics, src_Ks, cur_invK,
                    min_depth, max_depth):
    dp = _depth_planes(min_depth.reshape(-1)[0], max_depth.reshape(-1)[0])
    xcs, ycs = _coords(src_extrinsics, src_Ks, cur_invK, dp)
    srcw = np.ascontiguousarray(
        src_feats[0].transpose(2, 0, 1, 3)).astype(ml_dtypes.bfloat16)
    curb = np.ascontiguousarray(
        cur_feats[0].transpose(2, 0, 1)).astype(ml_dtypes.bfloat16)
    in_maps = []
    for k in range(N_CORES):
        sl = slice(k * DLOC, (k + 1) * DLOC)
        in_maps.append({
            "srcw": srcw,
            "curb": curb,
            "ay": _hat(ycs[:, sl], H, KPAD),
            "bx": _hat(xcs[:, sl], W),
        })
    return in_maps


def _run(inputs, trace=False):
    from concourse.bass_utils import run_bass_kernel_spmd
    nc = _get_nc()
    in_maps = _prepare_inputs(**inputs)
    res = run_bass_kernel_spmd(nc, in_maps, core_ids=list(range(N_CORES)),
                               trace=trace)
    # per-core result is (W, DLOC, H) -> (DLOC, H, W)
    parts = [res.results[k]["out"].transpose(1, 2, 0) for k in range(N_CORES)]
    out = np.concatenate(parts, 0)[None].astype(np.float32)
    return out, res


def kernel(cur_feats, src_feats, src_extrinsics, src_Ks, cur_invK,
           min_depth, max_depth):
    args = dict(cur_feats=np.asarray(cur_feats), src_feats=np.asarray(src_feats),
                src_extrinsics=np.asarray(src_extrinsics),
                src_Ks=np.asarray(src_Ks), cur_invK=np.asarray(cur_invK),
                min_depth=np.asarray(min_depth), max_depth=np.asarray(max_depth))
    if not _is_separable(args["src_extrinsics"], args["src_Ks"],
                         args["cur_invK"]):
        return _reference_numpy(**args)
    out, _ = _run(args)
    return out


# revision 65
# speedup vs baseline: 1.1911x; 1.1911x over previous
"""Plane-sweep cost-volume kernel for Trainium2 (8 NeuronCores).

Problem shape (hardcoded): B=1, V=4 source views, C=16 feature channels,
H=64, W=96, D=64 depth planes.  Output: (1, D, H, W) float32.

Strategy
--------
The benchmark geometry has identity rotations (extrinsics are pure
translations) and zero-skew pinhole intrinsics, so for each (view, depth
plane) the warp from output pixels to source-image sample coordinates is an
axis-separable affine map.  Bilinear grid_sample with zero padding then
factorizes exactly into two 1-D linear interpolations, each a small dense
matrix of "hat" functions hat(t - k) = max(0, 1 - |t - k|):

    warped_c = Ay(v,d) @ src_c @ Bx(v,d)^T

so the whole cost volume becomes TensorEngine matmuls — no gathers.  The
view sum is accumulated in PSUM, and the channel dot with cur_feats is a
vector multiply + reduce.

Optimizations vs the first working version (101.7us -> ~58us measured):
  * Ay/Bx hat matrices + bf16 features precomputed on host and DMAed on the
    two HWDGE queues, biggest-consumer-first (removes ~25us of on-device
    setup that serialized in front of the matmuls).
  * Stage-1 contraction zero-padded from K=64 to K=96 rows: with only half
    the PE array active, the PE_HAM activity monitor never un-throttles the
    2.4 GHz clock gate (stage 1 measured a flat 1.2 GHz); at K=96 it warms
    after ~3.4us.  Seven dummy matmuls on a scratch tile pre-warm the PE
    during the input-DMA wait.
  * The 3.1M-element stage-1 PSUM->SBUF intermediate is the hard wall (only
    vector + scalar can read PSUM, ~1.1-1.2us per 1024-elem chunk each);
    casts alternate engines, 4-deep PSUM buffering.
  * Channel dot: scalar (idle in stage 2) copies PSUM->SBUF bf16 so the
    multiply runs in the DVE 2x bf16 mode; reduction is a vector X-reduce or
    a vector+gpsimd hybrid add-tree per REDUCE_PATTERN (empirically tuned -
    moving the last plane's reduce to vector measures ~10us slower).

Sharding: depth planes across the 8 cores (8 planes each); features are
replicated.  If the inputs do not have the separable structure, we fall
back to an exact numpy implementation.
"""

import numpy as np
import ml_dtypes

H, W, D, V, C = 64, 96, 64, 4, 16
N_CORES = 8
DLOC = D // N_CORES            # 8 depth planes per core
EPS = 1e-8
OOB = 1.0e9                    # sample coord pushed out of range => zero weights

# cast-engine pattern for the 32 stage-1 cast chunks: 's'calar / 'v'ector
# (scalar is slightly faster per chunk, so it takes 17 of 32)
CAST_PATTERN = "svsvsvsvsvsvsvsssvsvsvsvsvsvsvsv"
# reduce-engine pattern for the 8 plane dots: 'v'ector / 'g'psimd
REDUCE_PATTERN = "gggggvvg"
# stage-1 contraction rows: H=64 padded with zero rows to 96.  With only 64
# of 128 PE rows active the PE_HAM activity monitor never un-throttles the
# clock gate (stage 1 measured a flat 1.2 GHz for 34 us); at K=96 the array
# reads as busy and reaches 2.4 GHz after ~3.4 us.
KPAD = 96
# Stage-1 matmuls convoy at cast speed (frequent small stalls), which keeps
# the PE HAM warm through the stage boundary on its own — no dummy matmuls
# needed (they would just delay stage 2 in the in-order queue).

_CACHE = {}


# --------------------------------------------------------------------------
# Device kernel
# --------------------------------------------------------------------------
def _build_nc():
    import concourse.bacc as bacc
    import concourse.tile as tile
    from concourse import mybir

    fp32 = mybir.dt.float32
    bf16 = mybir.dt.bfloat16
    Alu = mybir.AluOpType
    Axis = mybir.AxisListType

    nc = bacc.Bacc("TRN2", target_bir_lowering=False, debug=False,
                   num_devices=N_CORES)

    srcw = nc.dram_tensor("srcw", [H, V, C, W], bf16, kind="ExternalInput")
    ay = nc.dram_tensor("ay", [KPAD, V * DLOC * H], bf16, kind="ExternalInput")
    bx = nc.dram_tensor("bx", [W, V * DLOC * W], bf16, kind="ExternalInput")
    curb = nc.dram_tensor("curb", [W, C, H], bf16, kind="ExternalInput")
    out = nc.dram_tensor("out", [W, DLOC, H], fp32, kind="ExternalOutput")

    with tile.TileContext(nc) as tc:
        with (
            tc.tile_pool(name="consts", bufs=1) as consts,
            tc.tile_pool(name="tp", bufs=1) as tp_pool,
            tc.tile_pool(name="dot", bufs=3) as dot_pool,
            tc.tile_pool(name="osb", bufs=3) as out_pool,
        ):
            # ---- load constants (all pre-packed on host) ------------------
            # HWDGE queues only (sync + scalar), ordered so the view-0
            # operands land first and stage 1 can start ASAP.  Partition rows
            # H..KPAD-1 of the stage-1 operands are zero (see KPAD note).
            # scratch operand for PE warm-up matmuls (memset so the race
            # detector sees it initialized; values are never read)
            warmg = consts.tile([KPAD, DLOC * H], bf16, tag="warmg")
            nc.vector.memset(warmg, 0.0)
            src_all = consts.tile([KPAD, V, C, W], bf16, tag="src_all")
            # zero the K-pad rows with the two otherwise-idle engines while
            # the input DMAs run (the pad must be exactly zero: it multiplies
            # the Ay pad rows, which are zero via the host, but 0*garbage
            # could be NaN if this region held Inf/NaN bits)
            nc.vector.memset(src_all[H:KPAD, 0:2, :, :], 0.0)
            nc.gpsimd.memset(src_all[H:KPAD, 2:4, :, :], 0.0)
            nc.sync.dma_start(out=src_all[0:H, 0:1, :, :],
                              in_=srcw.ap()[:, 0:1, :, :])
            Ay = consts.tile([KPAD, V * DLOC * H], bf16, tag="ay")
            nc.scalar.dma_start(out=Ay[:, 0:DLOC * H],
                                in_=ay.ap()[:, 0:DLOC * H])
            nc.sync.dma_start(out=src_all[0:H, 1:2, :, :],
                              in_=srcw.ap()[:, 1:2, :, :])
            nc.scalar.dma_start(out=Ay[:, DLOC * H:],
                                in_=ay.ap()[:, DLOC * H:])
            nc.sync.dma_start(out=src_all[0:H, 2:4, :, :],
                              in_=srcw.ap()[:, 2:4, :, :])
            Bx = consts.tile([W, V * DLOC * W], bf16, tag="bx")
            nc.scalar.dma_start(out=Bx, in_=bx.ap())
            cur_t = consts.tile([W, C, H], bf16, tag="cur")
            nc.scalar.dma_start(out=cur_t, in_=curb.ap())
            src_t = [src_all[:, v, :, :] for v in range(V)]

            tps = [tp_pool.tile([W, C, DLOC * H], bf16, tag=f"tp{v}",
                                name=f"tp{v}")
                   for v in range(V)]

            # ---- stage 1: y-interpolation -------------------------------
            # tp_v(w; c, (d,py)) = sum_h src(h; c, w) * Ay(h; v, (d,py))
            ci = 0
            with tc.tile_pool(name="ps1", bufs=4, space="PSUM") as ps1_pool:
                # warm the PE HAM clock gate during the input-DMA wait: these
                # depend on nothing, so by the time the real operands land the
                # PE is already at 2.4 GHz instead of spending its first
                # ~3.4 us of real work at 1.2 GHz.
                ps1 = ps1_pool.tile([W, 2, DLOC * H], fp32)
                for _ in range(9):
                    nc.tensor.matmul(ps1[:, 0, :], warmg[:, 0:W], warmg,
                                     start=True, stop=True)
                for v in range(V):
                    rhs = Ay[:, v * DLOC * H:(v + 1) * DLOC * H]   # (64, 512)
                    for cq in range(C // 2):
                        ps1 = ps1_pool.tile([W, 2, DLOC * H], fp32)
                        for cc in range(2):
                            nc.tensor.matmul(
                                ps1[:, cc, :], src_t[v][:, cq * 2 + cc, :],
                                rhs, start=True, stop=True)
                        dst = tps[v][:, cq * 2:cq * 2 + 2, :]
                        if CAST_PATTERN[ci % len(CAST_PATTERN)] == 's':
                            nc.scalar.copy(dst, ps1)
                        else:
                            nc.vector.tensor_copy(dst, ps1)
                        ci += 1

            # ---- stage 2: x-interpolation + view accumulation -----------
            osb_all = out_pool.tile([W, DLOC, H], fp32, tag="osb_all")
            ps2_cm = tc.tile_pool(name="ps2", bufs=3, space="PSUM")
            ps2_pool = ps2_cm.__enter__()
            for d in range(DLOC):
                ps2 = ps2_pool.tile([W, C, H], fp32)
                for v in range(V):
                    lhsT = Bx[:, (v * DLOC + d) * W:(v * DLOC + d + 1) * W]
                    for hh in range(2):
                        nc.tensor.matmul(
                            ps2[:, hh * 8:hh * 8 + 8, :],
                            lhsT,
                            tps[v][:, hh * 8:hh * 8 + 8, d * H:(d + 1) * H],
                            start=(v == 0), stop=(v == V - 1))
                # channel dot with cur + write out.  For all but the last
                # plane, the (otherwise idle) scalar engine moves ps2 out of
                # PSUM so the multiply runs in the DVE 2x bf16 SBUF mode; the
                # last plane multiplies straight from PSUM to shorten the
                # tail dependency chain.
                m = dot_pool.tile([W, C, H], bf16)
                if d < DLOC - 1:
                    p2b = dot_pool.tile([W, C, H], bf16)
                    nc.scalar.copy(p2b, ps2)
                    nc.vector.tensor_mul(m, p2b, cur_t)
                else:
                    nc.vector.tensor_mul(m, ps2, cur_t)
                osb = osb_all[:, d, :]
                if REDUCE_PATTERN[d % len(REDUCE_PATTERN)] == 'v':
                    nc.vector.tensor_reduce(
                        osb, m.transpose([0, 2, 1]), axis=Axis.X, op=Alu.add)
                else:
                    # gpsimd can't X-reduce; binary tree of adds over C.
                    # The first (largest) level runs on vector in its 2x bf16
                    # mode; the small tail levels go to gpsimd (~2.1 cyc/elem)
                    # except on the last plane, where the cross-engine handoff
                    # latency would sit on the kernel's critical tail.
                    tail_eng = nc.vector if d == DLOC - 1 else nc.gpsimd
                    s1 = dot_pool.tile([W, C // 2, H], bf16)
                    nc.vector.tensor_add(s1, m[:, :8, :], m[:, 8:, :])
                    s2 = dot_pool.tile([W, C // 4, H], bf16)
                    tail_eng.tensor_add(s2, s1[:, :4, :], s1[:, 4:, :])
                    s3 = dot_pool.tile([W, C // 8, H], bf16)
                    tail_eng.tensor_add(s3, s2[:, :2, :], s2[:, 2:, :])
                    tail_eng.tensor_add(osb, s3[:, 0, :], s3[:, 1, :])
                if d == 3:
                    nc.sync.dma_start(out=out.ap()[:, 0:4, :],
                                      in_=osb_all[:, 0:4, :])
                elif d == 6:
                    nc.sync.dma_start(out=out.ap()[:, 4:7, :],
                                      in_=osb_all[:, 4:7, :])
                elif d == 7:
                    nc.sync.dma_start(out=out.ap()[:, 7:8, :],
                                      in_=osb_all[:, 7:8, :])
            ps2_cm.__exit__(None, None, None)

    nc.compile()
    return nc


def _get_nc():
    if "nc" not in _CACHE:
        _CACHE["nc"] = _build_nc()
    return _CACHE["nc"]


# --------------------------------------------------------------------------
# Host-side geometry
# --------------------------------------------------------------------------
def _depth_planes(min_depth, max_depth):
    """Mimic the reference's fp32 arithmetic."""
    ramp = np.linspace(0.0, 1.0, D, dtype=np.float32)
    inv_min = (np.float32(1.0) / np.float32(min_depth)).astype(np.float32)
    inv_max = (np.float32(1.0) / np.float32(max_depth)).astype(np.float32)
    return (np.float32(1.0) /
            (inv_min + (inv_max - inv_min) * ramp).astype(np.float32))


def _is_separable(src_extrinsics, src_Ks, cur_invK):
    E = src_extrinsics[0]          # (V,4,4)
    K = src_Ks[0]                  # (V,4,4)
    iK = cur_invK[0]               # (4,4)
    eye3 = np.eye(3, dtype=E.dtype)
    for v in range(V):
        if not np.array_equal(E[v, :3, :3], eye3):
            return False
        if not np.array_equal(E[v, 3], np.array([0, 0, 0, 1], dtype=E.dtype)):
            return False
        k = K[v]
        if not (k[0, 1] == 0 and k[0, 3] == 0 and k[1, 0] == 0 and k[1, 3] == 0
                and np.array_equal(k[2], np.array([0, 0, 1, 0], dtype=K.dtype))):
            return False
    if not (iK[0, 1] == 0 and iK[1, 0] == 0 and iK[2, 0] == 0
            and iK[2, 1] == 0 and iK[2, 2] == 1):
        return False
    return True


def _coords(src_extrinsics, src_Ks, cur_invK, depths):
    """Per-(view, plane) 1-D sample coordinates: x[v,d,px], y[v,d,py]."""
    E = src_extrinsics[0].astype(np.float64)
    K = src_Ks[0].astype(np.float64)
    iK = cur_invK[0].astype(np.float64)
    i00, i02 = iK[0, 0], iK[0, 2]
    i11, i12 = iK[1, 1], iK[1, 2]
    px = np.arange(W, dtype=np.float64) + 0.5
    py = np.arange(H, dtype=np.float64) + 0.5
    xcs = np.empty((V, D, W), np.float64)
    ycs = np.empty((V, D, H), np.float64)
    for v in range(V):
        k00, k02 = K[v, 0, 0], K[v, 0, 2]
        k11, k12 = K[v, 1, 1], K[v, 1, 2]
        tx, ty, tz = E[v, 0, 3], E[v, 1, 3], E[v, 2, 3]
        for d in range(D):
            Dd = float(depths[d])
            z32 = np.float32(depths[d]) + np.float32(tz)        # ref fp32 z
            if not (z32 > 0):
                xcs[v, d] = OOB
                ycs[v, d] = OOB
                continue
            Zs = float(np.float32(z32 + np.float32(EPS)))
            rx = i00 * px + i02
            ry = i11 * py + i12
            u = (k00 * rx * Dd + k02 * Dd + k00 * tx + k02 * tz) / Zs
            vv = (k11 * ry * Dd + k12 * Dd + k11 * ty + k12 * tz) / Zs
            xcs[v, d] = np.clip(np.nan_to_num(u - 0.5, nan=OOB,
                                              posinf=OOB, neginf=-OOB),
                                -OOB, OOB)
            ycs[v, d] = np.clip(np.nan_to_num(vv - 0.5, nan=OOB,
                                              posinf=OOB, neginf=-OOB),
                                -OOB, OOB)
    return xcs, ycs


def _hat(coords, n, npad=None):
    """coords: (V, DLOC, M) sample positions -> (npad, V*DLOC*M) bf16 hat
    matrix, hat[k, (v,d,m)] = relu(1 - |coords[v,d,m] - k|), zero rows
    beyond n."""
    kk = np.arange(n, dtype=np.float64)
    h = np.maximum(0.0, 1.0 - np.abs(coords[..., None, :] -
                                     kk[None, None, :, None]))
    # h: (V, DLOC, n_k, M) -> (n_k, V, DLOC, M)
    h = np.ascontiguousarray(h.transpose(2, 0, 1, 3)).reshape(n, -1)
    if npad is not None and npad > n:
        h = np.concatenate([h, np.zeros((npad - n, h.shape[1]), h.dtype)], 0)
    return h.astype(ml_dtypes.bfloat16)


# --------------------------------------------------------------------------
# Exact numpy fallback (general geometry)
# --------------------------------------------------------------------------
def _reference_numpy(cur_feats, src_feats, src_extrinsics, src_Ks, cur_invK,
                     min_depth, max_depth):
    f32 = np.float32
    N = H * W
    dp = _depth_planes(min_depth.reshape(-1)[0], max_depth.reshape(-1)[0])
    xx, yy = np.meshgrid(np.arange(W, dtype=f32) + 0.5,
                         np.arange(H, dtype=f32) + 0.5)
    pix = np.stack([xx.ravel(), yy.ravel(), np.ones(N, f32)], 0)       # (3,N)
    rays = cur_invK[0, :3, :3].astype(f32) @ pix                       # (3,N)
    world = rays[None] * dp[:, None, None]                             # (D,3,N)
    world4 = np.concatenate([world, np.ones((D, 1, N), f32)], 1)       # (D,4,N)
    P = np.einsum("vij,vjk->vik", src_Ks[0], src_extrinsics[0])[:, :3]  # (V,3,4)
    cam = np.einsum("vij,djn->vdin", P, world4).astype(f32)            # (V,D,3,N)
    z = cam[:, :, 2]
    u = cam[:, :, 0] / (z + f32(EPS))
    vv = cam[:, :, 1] / (z + f32(EPS))
    x = (u - 0.5).astype(f32).reshape(V, D * N)
    y = (vv - 0.5).astype(f32).reshape(V, D * N)
    out = np.zeros((D, H, W), f32)
    cur = cur_feats[0].reshape(C, N)                                   # (C,N)
    for v in range(V):
        f = src_feats[0, v].reshape(C, N)
        x0 = np.floor(x[v])
        y0 = np.floor(y[v])
        acc = np.zeros((C, D * N), f32)
        for dx in (0.0, 1.0):
            for dy in (0.0, 1.0):
                xi = x0 + dx
                yi = y0 + dy
                wgt = (1.0 - np.abs(x[v] - xi)) * (1.0 - np.abs(y[v] - yi))
                valid = ((xi >= 0) & (xi < W) & (yi >= 0) & (yi < H))
                idx = (np.clip(yi, 0, H - 1) * W +
                       np.clip(xi, 0, W - 1)).astype(np.int64)
                acc += f[:, idx] * (wgt * valid.astype(f32))[None]
        dot = (acc.reshape(C, D, N) *
               cur[:, None, :]).sum(0)                                 # (D,N)
        mask = (z[v] > 0).astype(f32)                                  # (D,N)
        out += (dot * mask).reshape(D, H, W)
    return out[None].astype(np.float32)


# --------------------------------------------------------------------------
# Entry points
# --------------------------------------------------------------------------
def _prepare_inputs(cur_feats, src_feats, src_extrinsics, src_Ks, cur_invK,
                    min_depth, max_depth):
    dp = _depth_planes(min_depth.reshape(-1)[0], max_depth.reshape(-1)[0])
    xcs, ycs = _coords(src_extrinsics, src_Ks, cur_invK, dp)
    srcw = np.ascontiguousarray(
        src_feats[0].transpose(2, 0, 1, 3)).astype(ml_dtypes.bfloat16)
    curb = np.ascontiguousarray(
        cur_feats[0].transpose(2, 0, 1)).astype(ml_dtypes.bfloat16)
    in_maps = []
    for k in range(N_CORES):
        sl = slice(k * DLOC, (k + 1) * DLOC)
        in_maps.append({
            "srcw": srcw,
            "curb": curb,
            "ay": _hat(ycs[:, sl], H, KPAD),
            "bx": _hat(xcs[:, sl], W),
        })
    return in_maps


def _run(inputs, trace=False):
    from concourse.bass_utils import run_bass_kernel_spmd
    nc = _get_nc()
    in_maps = _prepare_inputs(**inputs)
    res = run_bass_kernel_spmd(nc, in_maps, core_ids=list(range(N_CORES)),
                               trace=trace)
    # per-core result is (W, DLOC, H) -> (DLOC, H, W)
    parts = [res.results[k]["out"].transpose(1, 2, 0) for k in range(N_CORES)]
    out = np.concatenate(parts, 0)[None].astype(np.float32)
    return out, res


def kernel(cur_feats, src_feats, src_extrinsics, src_Ks, cur_invK,
           min_depth, max_depth):
    args = dict(cur_feats=np.asarray(cur_feats), src_feats=np.asarray(src_feats),
                src_extrinsics=np.asarray(src_extrinsics),
                src_Ks=np.asarray(src_Ks), cur_invK=np.asarray(cur_invK),
                min_depth=np.asarray(min_depth), max_depth=np.asarray(max_depth))
    if not _is_separable(args["src_extrinsics"], args["src_Ks"],
                         args["cur_invK"]):
        return _reference_numpy(**args)
    out, _ = _run(args)
    return out


# revision 70
# speedup vs baseline: 1.2892x; 1.0823x over previous
"""Plane-sweep cost-volume kernel for Trainium2 (8 NeuronCores).

Problem shape (hardcoded): B=1, V=4 source views, C=16 feature channels,
H=64, W=96, D=64 depth planes.  Output: (1, D, H, W) float32.

Strategy
--------
The benchmark geometry has identity rotations (extrinsics are pure
translations) and zero-skew pinhole intrinsics, so for each (view, depth
plane) the warp from output pixels to source-image sample coordinates is an
axis-separable affine map.  Bilinear grid_sample with zero padding then
factorizes exactly into two 1-D linear interpolations, each a small dense
matrix of "hat" functions hat(t - k) = max(0, 1 - |t - k|):

    warped_c = Ay(v,d) @ src_c @ Bx(v,d)^T

so the whole cost volume becomes TensorEngine matmuls — no gathers.  The
view sum is accumulated in PSUM, and the channel dot with cur_feats is a
vector multiply + reduce.

Optimizations vs the first working version (101.7us -> ~58us measured):
  * Ay/Bx hat matrices + bf16 features precomputed on host and DMAed on the
    two HWDGE queues, biggest-consumer-first (removes ~25us of on-device
    setup that serialized in front of the matmuls).
  * Stage-1 contraction zero-padded from K=64 to K=96 rows: with only half
    the PE array active, the PE_HAM activity monitor never un-throttles the
    2.4 GHz clock gate (stage 1 measured a flat 1.2 GHz); at K=96 it warms
    after ~3.4us.  Seven dummy matmuls on a scratch tile pre-warm the PE
    during the input-DMA wait.
  * The 3.1M-element stage-1 PSUM->SBUF intermediate is the hard wall (only
    vector + scalar can read PSUM, ~1.1-1.2us per 1024-elem chunk each);
    casts alternate engines, 4-deep PSUM buffering.
  * Channel dot: scalar (idle in stage 2) copies PSUM->SBUF bf16 so the
    multiply runs in the DVE 2x bf16 mode; reduction is a vector X-reduce or
    a vector+gpsimd hybrid add-tree per REDUCE_PATTERN (empirically tuned -
    moving the last plane's reduce to vector measures ~10us slower).

Sharding: depth planes across the 8 cores (8 planes each); features are
replicated.  If the inputs do not have the separable structure, we fall
back to an exact numpy implementation.
"""

import numpy as np
import ml_dtypes

H, W, D, V, C = 64, 96, 64, 4, 16
N_CORES = 8
DLOC = D // N_CORES            # 8 depth planes per core
EPS = 1e-8
OOB = 1.0e9                    # sample coord pushed out of range => zero weights

# cast-engine pattern for the 32 stage-1 cast chunks: 's'calar / 'v'ector
# (scalar is slightly faster per chunk, so it takes 17 of 32)
CAST_PATTERN = "svsvsvsvsvsvsvsssvsvsvsvsvsvsvsv"
# reduce-engine pattern for the 8 plane dots: 'v'ector / 'g'psimd
REDUCE_PATTERN = "gggggvvg"
# stage-1 contraction rows: H=64 padded with zero rows to 96.  With only 64
# of 128 PE rows active the PE_HAM activity monitor never un-throttles the
# clock gate (stage 1 measured a flat 1.2 GHz for 34 us); at K=96 the array
# reads as busy and reaches 2.4 GHz after ~3.4 us.
KPAD = 96
# Stage-1 matmuls convoy at cast speed (frequent small stalls), which keeps
# the PE HAM warm through the stage boundary on its own — no dummy matmuls
# needed (they would just delay stage 2 in the in-order queue).

_CACHE = {}


# --------------------------------------------------------------------------
# Device kernel
# --------------------------------------------------------------------------
def _build_nc():
    import concourse.bacc as bacc
    import concourse.tile as tile
    from concourse import mybir

    fp32 = mybir.dt.float32
    bf16 = mybir.dt.bfloat16
    Alu = mybir.AluOpType
    Axis = mybir.AxisListType

    nc = bacc.Bacc("TRN2", target_bir_lowering=False, debug=False,
                   num_devices=N_CORES)

    srcw = nc.dram_tensor("srcw", [H, V, C, W], bf16, kind="ExternalInput")
    ay = nc.dram_tensor("ay", [KPAD, V * DLOC * H], bf16, kind="ExternalInput")
    bx = nc.dram_tensor("bx", [W, V * DLOC * W], bf16, kind="ExternalInput")
    curb = nc.dram_tensor("curb", [W, C, H], bf16, kind="ExternalInput")
    out = nc.dram_tensor("out", [W, DLOC, H], fp32, kind="ExternalOutput")

    with tile.TileContext(nc) as tc:
        with (
            tc.tile_pool(name="consts", bufs=1) as consts,
            tc.tile_pool(name="tp", bufs=1) as tp_pool,
            tc.tile_pool(name="dot", bufs=3) as dot_pool,
            tc.tile_pool(name="osb", bufs=3) as out_pool,
        ):
            # ---- load constants (all pre-packed on host) ------------------
            # HWDGE queues only (sync + scalar), ordered so the view-0
            # operands land first and stage 1 can start ASAP.  Partition rows
            # H..KPAD-1 of the stage-1 operands are zero (see KPAD note).
            # scratch operand for PE warm-up matmuls (memset so the race
            # detector sees it initialized; values are never read)
            warmg = consts.tile([KPAD, DLOC * H], bf16, tag="warmg")
            nc.vector.memset(warmg, 0.0)
            src_all = consts.tile([KPAD, V, C, W], bf16, tag="src_all")
            # zero the K-pad rows with the two otherwise-idle engines while
            # the input DMAs run (the pad must be exactly zero: it multiplies
            # the Ay pad rows, which are zero via the host, but 0*garbage
            # could be NaN if this region held Inf/NaN bits)
            nc.vector.memset(src_all[H:KPAD, 0:2, :, :], 0.0)
            nc.gpsimd.memset(src_all[H:KPAD, 2:4, :, :], 0.0)
            nc.sync.dma_start(out=src_all[0:H, 0:1, :, :],
                              in_=srcw.ap()[:, 0:1, :, :])
            Ay = consts.tile([KPAD, V * DLOC * H], bf16, tag="ay")
            nc.scalar.dma_start(out=Ay[:, 0:DLOC * H],
                                in_=ay.ap()[:, 0:DLOC * H])
            nc.sync.dma_start(out=src_all[0:H, 1:2, :, :],
                              in_=srcw.ap()[:, 1:2, :, :])
            nc.scalar.dma_start(out=Ay[:, DLOC * H:],
                                in_=ay.ap()[:, DLOC * H:])
            nc.sync.dma_start(out=src_all[0:H, 2:4, :, :],
                              in_=srcw.ap()[:, 2:4, :, :])
            Bx = consts.tile([W, V * DLOC * W], bf16, tag="bx")
            nc.scalar.dma_start(out=Bx, in_=bx.ap())
            cur_t = consts.tile([W, C, H], bf16, tag="cur")
            nc.scalar.dma_start(out=cur_t, in_=curb.ap())
            src_t = [src_all[:, v, :, :] for v in range(V)]

            tps = [tp_pool.tile([W, C, DLOC * H], bf16, tag=f"tp{v}",
                                name=f"tp{v}")
                   for v in range(V)]

            # ---- stage 1: y-interpolation -------------------------------
            # tp_v(w; c, (d,py)) = sum_h src(h; c, w) * Ay(h; v, (d,py))
            ci = 0
            # one PSUM pool for BOTH stages (same tile shape): a separate
            # stage-2 pool would reuse stage-1's banks and its first write
            # would barrier on ALL 32 casts; sharing the pool lets stage 2
            # start while the last cast chunks drain.
            ps_cm = tc.tile_pool(name="ps", bufs=4, space="PSUM")
            ps1_pool = ps_cm.__enter__()
            if True:
                # warm the PE HAM clock gate during the input-DMA wait: these
                # depend on nothing, so by the time the real operands land the
                # PE is already at 2.4 GHz instead of spending its first
                # ~3.4 us of real work at 1.2 GHz.
                ps1 = ps1_pool.tile([W, 2, DLOC * H], fp32)
                for _ in range(9):
                    nc.tensor.matmul(ps1[:, 0, :], warmg[:, 0:W], warmg,
                                     start=True, stop=True)
                for v in range(V):
                    rhs = Ay[:, v * DLOC * H:(v + 1) * DLOC * H]   # (64, 512)
                    for cq in range(C // 2):
                        ps1 = ps1_pool.tile([W, 2, DLOC * H], fp32)
                        for cc in range(2):
                            nc.tensor.matmul(
                                ps1[:, cc, :], src_t[v][:, cq * 2 + cc, :],
                                rhs, start=True, stop=True)
                        dst = tps[v][:, cq * 2:cq * 2 + 2, :]
                        if CAST_PATTERN[ci % len(CAST_PATTERN)] == 's':
                            nc.scalar.copy(dst, ps1)
                        else:
                            nc.vector.tensor_copy(dst, ps1)
                        ci += 1

            # ---- stage 2: x-interpolation + view accumulation -----------
            osb_all = out_pool.tile([W, DLOC, H], fp32, tag="osb_all")
            for d in range(DLOC):
                # same allocated shape as the stage-1 tiles; [:, hh, :] is
                # the (8 channels x 64 py) = 512-element bank-aligned half
                ps2 = ps1_pool.tile([W, 2, DLOC * H], fp32, name="ps1")
                for v in range(V):
                    lhsT = Bx[:, (v * DLOC + d) * W:(v * DLOC + d + 1) * W]
                    for hh in range(2):
                        nc.tensor.matmul(
                            ps2[:, hh, :],
                            lhsT,
                            tps[v][:, hh * 8:hh * 8 + 8, d * H:(d + 1) * H],
                            start=(v == 0), stop=(v == V - 1))
                ps2f = ps2.rearrange("p a b -> p (a b)")
                # channel dot with cur + write out.  For all but the last
                # plane, the (otherwise idle) scalar engine moves ps2 out of
                # PSUM so the multiply runs in the DVE 2x bf16 SBUF mode; the
                # last plane multiplies straight from PSUM to shorten the
                # tail dependency chain.
                m = dot_pool.tile([W, C, H], bf16)
                if d < DLOC - 1:
                    p2b = dot_pool.tile([W, C, H], bf16)
                    nc.scalar.copy(p2b.rearrange("p c h -> p (c h)"), ps2f)
                    nc.vector.tensor_mul(m, p2b, cur_t)
                else:
                    nc.vector.tensor_mul(m.rearrange("p c h -> p (c h)"),
                                         ps2f,
                                         cur_t.rearrange("p c h -> p (c h)"))
                osb = osb_all[:, d, :]
                if REDUCE_PATTERN[d % len(REDUCE_PATTERN)] == 'v':
                    nc.vector.tensor_reduce(
                        osb, m.transpose([0, 2, 1]), axis=Axis.X, op=Alu.add)
                else:
                    # gpsimd can't X-reduce; binary tree of adds over C.
                    # The first (largest) level runs on vector in its 2x bf16
                    # mode; the small tail levels go to gpsimd (~2.1 cyc/elem)
                    # except on the last plane, where the cross-engine handoff
                    # latency would sit on the kernel's critical tail.
                    tail_eng = nc.vector if d == DLOC - 1 else nc.gpsimd
                    s1 = dot_pool.tile([W, C // 2, H], bf16)
                    nc.vector.tensor_add(s1, m[:, :8, :], m[:, 8:, :])
                    s2 = dot_pool.tile([W, C // 4, H], bf16)
                    tail_eng.tensor_add(s2, s1[:, :4, :], s1[:, 4:, :])
                    s3 = dot_pool.tile([W, C // 8, H], bf16)
                    tail_eng.tensor_add(s3, s2[:, :2, :], s2[:, 2:, :])
                    tail_eng.tensor_add(osb, s3[:, 0, :], s3[:, 1, :])
                if d == 3:
                    nc.sync.dma_start(out=out.ap()[:, 0:4, :],
                                      in_=osb_all[:, 0:4, :])
                elif d == 6:
                    nc.sync.dma_start(out=out.ap()[:, 4:7, :],
                                      in_=osb_all[:, 4:7, :])
                elif d == 7:
                    nc.sync.dma_start(out=out.ap()[:, 7:8, :],
                                      in_=osb_all[:, 7:8, :])
            ps_cm.__exit__(None, None, None)

    nc.compile()
    return nc


def _get_nc():
    if "nc" not in _CACHE:
        _CACHE["nc"] = _build_nc()
    return _CACHE["nc"]


# --------------------------------------------------------------------------
# Host-side geometry
# --------------------------------------------------------------------------
def _depth_planes(min_depth, max_depth):
    """Mimic the reference's fp32 arithmetic."""
    ramp = np.linspace(0.0, 1.0, D, dtype=np.float32)
    inv_min = (np.float32(1.0) / np.float32(min_depth)).astype(np.float32)
    inv_max = (np.float32(1.0) / np.float32(max_depth)).astype(np.float32)
    return (np.float32(1.0) /
            (inv_min + (inv_max - inv_min) * ramp).astype(np.float32))


def _is_separable(src_extrinsics, src_Ks, cur_invK):
    E = src_extrinsics[0]          # (V,4,4)
    K = src_Ks[0]                  # (V,4,4)
    iK = cur_invK[0]               # (4,4)
    eye3 = np.eye(3, dtype=E.dtype)
    for v in range(V):
        if not np.array_equal(E[v, :3, :3], eye3):
            return False
        if not np.array_equal(E[v, 3], np.array([0, 0, 0, 1], dtype=E.dtype)):
            return False
        k = K[v]
        if not (k[0, 1] == 0 and k[0, 3] == 0 and k[1, 0] == 0 and k[1, 3] == 0
                and np.array_equal(k[2], np.array([0, 0, 1, 0], dtype=K.dtype))):
            return False
    if not (iK[0, 1] == 0 and iK[1, 0] == 0 and iK[2, 0] == 0
            and iK[2, 1] == 0 and iK[2, 2] == 1):
        return False
    return True


def _coords(src_extrinsics, src_Ks, cur_invK, depths):
    """Per-(view, plane) 1-D sample coordinates: x[v,d,px], y[v,d,py]."""
    E = src_extrinsics[0].astype(np.float64)
    K = src_Ks[0].astype(np.float64)
    iK = cur_invK[0].astype(np.float64)
    i00, i02 = iK[0, 0], iK[0, 2]
    i11, i12 = iK[1, 1], iK[1, 2]
    px = np.arange(W, dtype=np.float64) + 0.5
    py = np.arange(H, dtype=np.float64) + 0.5
    xcs = np.empty((V, D, W), np.float64)
    ycs = np.empty((V, D, H), np.float64)
    for v in range(V):
        k00, k02 = K[v, 0, 0], K[v, 0, 2]
        k11, k12 = K[v, 1, 1], K[v, 1, 2]
        tx, ty, tz = E[v, 0, 3], E[v, 1, 3], E[v, 2, 3]
        for d in range(D):
            Dd = float(depths[d])
            z32 = np.float32(depths[d]) + np.float32(tz)        # ref fp32 z
            if not (z32 > 0):
                xcs[v, d] = OOB
                ycs[v, d] = OOB
                continue
            Zs = float(np.float32(z32 + np.float32(EPS)))
            rx = i00 * px + i02
            ry = i11 * py + i12
            u = (k00 * rx * Dd + k02 * Dd + k00 * tx + k02 * tz) / Zs
            vv = (k11 * ry * Dd + k12 * Dd + k11 * ty + k12 * tz) / Zs
            xcs[v, d] = np.clip(np.nan_to_num(u - 0.5, nan=OOB,
                                              posinf=OOB, neginf=-OOB),
                                -OOB, OOB)
            ycs[v, d] = np.clip(np.nan_to_num(vv - 0.5, nan=OOB,
                                              posinf=OOB, neginf=-OOB),
                                -OOB, OOB)
    return xcs, ycs


def _hat(coords, n, npad=None):
    """coords: (V, DLOC, M) sample positions -> (npad, V*DLOC*M) bf16 hat
    matrix, hat[k, (v,d,m)] = relu(1 - |coords[v,d,m] - k|), zero rows
    beyond n."""
    kk = np.arange(n, dtype=np.float64)
    h = np.maximum(0.0, 1.0 - np.abs(coords[..., None, :] -
                                     kk[None, None, :, None]))
    # h: (V, DLOC, n_k, M) -> (n_k, V, DLOC, M)
    h = np.ascontiguousarray(h.transpose(2, 0, 1, 3)).reshape(n, -1)
    if npad is not None and npad > n:
        h = np.concatenate([h, np.zeros((npad - n, h.shape[1]), h.dtype)], 0)
    return h.astype(ml_dtypes.bfloat16)


# --------------------------------------------------------------------------
# Exact numpy fallback (general geometry)
# --------------------------------------------------------------------------
def _reference_numpy(cur_feats, src_feats, src_extrinsics, src_Ks, cur_invK,
                     min_depth, max_depth):
    f32 = np.float32
    N = H * W
    dp = _depth_planes(min_depth.reshape(-1)[0], max_depth.reshape(-1)[0])
    xx, yy = np.meshgrid(np.arange(W, dtype=f32) + 0.5,
                         np.arange(H, dtype=f32) + 0.5)
    pix = np.stack([xx.ravel(), yy.ravel(), np.ones(N, f32)], 0)       # (3,N)
    rays = cur_invK[0, :3, :3].astype(f32) @ pix                       # (3,N)
    world = rays[None] * dp[:, None, None]                             # (D,3,N)
    world4 = np.concatenate([world, np.ones((D, 1, N), f32)], 1)       # (D,4,N)
    P = np.einsum("vij,vjk->vik", src_Ks[0], src_extrinsics[0])[:, :3]  # (V,3,4)
    cam = np.einsum("vij,djn->vdin", P, world4).astype(f32)            # (V,D,3,N)
    z = cam[:, :, 2]
    u = cam[:, :, 0] / (z + f32(EPS))
    vv = cam[:, :, 1] / (z + f32(EPS))
    x = (u - 0.5).astype(f32).reshape(V, D * N)
    y = (vv - 0.5).astype(f32).reshape(V, D * N)
    out = np.zeros((D, H, W), f32)
    cur = cur_feats[0].reshape(C, N)                                   # (C,N)
    for v in range(V):
        f = src_feats[0, v].reshape(C, N)
        x0 = np.floor(x[v])
        y0 = np.floor(y[v])
        acc = np.zeros((C, D * N), f32)
        for dx in (0.0, 1.0):
            for dy in (0.0, 1.0):
                xi = x0 + dx
                yi = y0 + dy
                wgt = (1.0 - np.abs(x[v] - xi)) * (1.0 - np.abs(y[v] - yi))
                valid = ((xi >= 0) & (xi < W) & (yi >= 0) & (yi < H))
                idx = (np.clip(yi, 0, H - 1) * W +
                       np.clip(xi, 0, W - 1)).astype(np.int64)
                acc += f[:, idx] * (wgt * valid.astype(f32))[None]
        dot = (acc.reshape(C, D, N) *
               cur[:, None, :]).sum(0)                                 # (D,N)
        mask = (z[v] > 0).astype(f32)                                  # (D,N)
        out += (dot * mask).reshape(D, H, W)
    return out[None].astype(np.float32)


# --------------------------------------------------------------------------
# Entry points
# --------------------------------------------------------------------------
def _prepare_inputs(cur_feats, src_feats, src_extrinsics, src_Ks, cur_invK,
                    min_depth, max_depth):
    dp = _depth_planes(min_depth.reshape(-1)[0], max_depth.reshape(-1)[0])
    xcs, ycs = _coords(src_extrinsics, src_Ks, cur_invK, dp)
    srcw = np.ascontiguousarray(
        src_feats[0].transpose(2, 0, 1, 3)).astype(ml_dtypes.bfloat16)
    curb = np.ascontiguousarray(
        cur_feats[0].transpose(2, 0, 1)).astype(ml_dtypes.bfloat16)
    in_maps = []
    for k in range(N_CORES):
        sl = slice(k * DLOC, (k + 1) * DLOC)
        in_maps.append({
            "srcw": srcw,
            "curb": curb,
            "ay": _hat(ycs[:, sl], H, KPAD),
            "bx": _hat(xcs[:, sl], W),
        })
    return in_maps


def _run(inputs, trace=False):
    from concourse.bass_utils import run_bass_kernel_spmd
    nc = _get_nc()
    in_maps = _prepare_inputs(**inputs)
    res = run_bass_kernel_spmd(nc, in_maps, core_ids=list(range(N_CORES)),
                               trace=trace)
    # per-core result is (W, DLOC, H) -> (DLOC, H, W)
    parts = [res.results[k]["out"].transpose(1, 2, 0) for k in range(N_CORES)]
    out = np.concatenate(parts, 0)[None].astype(np.float32)
    return out, res


def kernel(cur_feats, src_feats, src_extrinsics, src_Ks, cur_invK,
           min_depth, max_depth):
    args = dict(cur_feats=np.asarray(cur_feats), src_feats=np.asarray(src_feats),
                src_extrinsics=np.asarray(src_extrinsics),
                src_Ks=np.asarray(src_Ks), cur_invK=np.asarray(cur_invK),
                min_depth=np.asarray(min_depth), max_depth=np.asarray(max_depth))
    if not _is_separable(args["src_extrinsics"], args["src_Ks"],
                         args["cur_invK"]):
        return _reference_numpy(**args)
    out, _ = _run(args)
    return out
